# revision 55
# baseline (speedup 1.0000x reference)
"""DiffuseRouter kernel for 8 TRN2 NeuronCores.

Reference computation (enable_time=False, soft_time_routing=True):
    out[b, l, d] = (1/3) * sum_g sum_e expert_emb_g[e, b, l, d]
i.e. a uniform-weighted sum of 28 expert planes per batch element.

Sharding: pure data-parallel over batch B=8 -> one batch element per core.
Each core streams its 28 [256, 1280] f32 planes (36.7 MB) across both HWDGE
rings at ~415 GB/s and reduces them on the fly.  The reduction is split by
free-dim columns across two engines so neither falls behind the stream
(an all-DVE chain lags ~11 us):

  - cols [0:1536):    DVE chain (fp32 scalar_tensor_tensor, 1/3 folded in)
  - cols [1536:2560): TensorE identity-matmul accumulation into 2 PSUM
                      banks (fp32r moving operand), evacuated at the end by
                      ScalarE with the 1/3 scale.

Planes 0..25 load as single 1.31 MB transfers (best stream bandwidth);
plane 26 splits into DVE/PE stripes and plane 27 into five shrinking pieces
so each engine's final op and the output stores fire progressively as
pieces land.  Finer piecing of the last planes measures *worse*: DVE is the
serial resource at the tail and per-piece op overhead adds to its backlog.
"""

import numpy as np

import concourse.bacc as bacc
import concourse.tile as tile
from concourse import mybir
from concourse.alu_op_type import AluOpType
from concourse.bass_utils import run_bass_kernel_spmd

N_CORES = 8
E_TOTAL = 28  # 4 + 8 + 16 experts across the 3 granularity levels
L, D = 256, 1280
P = 128  # SBUF partitions
FD = (L // P) * D  # 2560 free-dim elements per partition
SCALE = 1.0 / 3.0

BANK = 512  # one PSUM bank = 512 fp32
W_V = 1536  # DVE stripe width (cols [0:W_V))
NB = (FD - W_V) // BANK  # PSUM banks for the PE stripe
PE_F32R = True  # fp32r moving operand (1 cyc/row) vs fp32 (4 cyc/row)

_NC_CACHE = None


def _build_nc():
    nc = bacc.Bacc(
        "TRN2", target_bir_lowering=False, debug=False, enable_partition_id=False
    )
    x = nc.dram_tensor("x", [E_TOTAL, L, D], mybir.dt.float32, kind="ExternalInput")
    ident = nc.dram_tensor("ident", [P, P], mybir.dt.float32, kind="ExternalInput")
    out = nc.dram_tensor("out", [L, D], mybir.dt.float32, kind="ExternalOutput")

    # [E, 256, 1280] -> [E, 128, 2560]: partition p holds rows 2p, 2p+1
    # (contiguous 10240 B per partition -> fully linear 1.31 MB DMA per plane).
    x_t = x.ap().rearrange("e (p a) d -> e p (a d)", a=2)
    out_t = out.ap().rearrange("(p a) d -> p (a d)", a=2)

    f32 = mybir.dt.float32
    f32r = mybir.dt.float32r
    mult, add = AluOpType.mult, AluOpType.add
    last = E_TOTAL - 1

    # walrus requires fp32r-matmul operands to be *produced* as fp32r, so the
    # plane tiles are declared fp32r (the DMA is a pure 4-byte copy either
    # way) and the DVE reads its stripe bitcast back to fp32.
    pe_dt = f32r if PE_F32R else f32

    def dma_src(ap):
        return ap.bitcast(f32r) if PE_F32R else ap

    with tile.TileContext(nc) as tc:
        with (
            tc.tile_pool(name="ina", bufs=2) as pina,
            tc.tile_pool(name="inb", bufs=8) as pinb,
            tc.tile_pool(name="pieces", bufs=1) as ppiece,
            tc.tile_pool(name="single", bufs=1) as psingle,
            tc.tile_pool(name="psum", bufs=1, space="PSUM") as ppsum,
        ):
            ident_sb = psingle.tile([P, P], pe_dt, name="ident", tag="ident")
            acc = psingle.tile([P, W_V], f32, name="acc", tag="acc")
            stage = [
                psingle.tile([P, BANK], f32, name=f"stage{b}", tag=f"stage{b}")
                for b in range(NB)
            ]
            banks = [
                ppsum.tile([P, BANK], f32, name=f"bank{b}", tag=f"bank{b}")
                for b in range(NB)
            ]

            # Identity for the PE accumulation; ACT ring so it never queues
            # ahead of plane loads on the SP ring.
            nc.scalar.dma_start(out=ident_sb[:], in_=dma_src(ident.ap()))

            for e in range(last):
                # Planes 0..25: one full 1.31 MB transfer (best stream BW).
                # Plane 26 splits into DVE-stripe + PE-stripe pieces so the
                # DVE add fires ~1.3 us earlier -- only the last planes'
                # latency matters for the tail.
                ring = nc.sync if e % 2 == 0 else nc.scalar
                if e < last - 1:
                    t = pinb.tile([P, FD], pe_dt)
                    ring.dma_start(out=t[:], in_=dma_src(x_t[e]))
                    ta = t[:, 0:W_V].bitcast(f32)
                    tb = t[:, W_V:FD]
                else:
                    ta_t = pina.tile([P, W_V], f32)
                    ring.dma_start(out=ta_t[:], in_=x_t[e][:, 0:W_V])
                    tb_t = pina.tile([P, FD - W_V], pe_dt)
                    ring.dma_start(out=tb_t[:], in_=dma_src(x_t[e][:, W_V:FD]))
                    ta, tb = ta_t[:], tb_t[:]
                # DVE stripe: scale folded into every op (stt costs the same
                # as a plain tensor_tensor add at fp32 1x mode), so nothing
                # but the per-chunk stt remains after the last plane lands.
                if e == 0:
                    nc.vector.tensor_scalar_mul(acc[:], ta, SCALE)
                else:
                    nc.vector.scalar_tensor_tensor(
                        acc[:], ta, SCALE, acc[:], mult, add
                    )
                # PE stripe: accumulate into PSUM banks (plane 26's MMs are
                # emitted in the tail, interleaved per bank with the stop
                # MMs, so each bank's evacuation fires as early as possible).
                if e < last - 1:
                    for b in range(NB):
                        nc.tensor.matmul(
                            banks[b][:],
                            ident_sb[:],
                            tb[:, b * BANK : (b + 1) * BANK],
                            start=(e == 0),
                            stop=False,
                        )
                else:
                    tb26 = tb

            # Last plane in pieces: PE banks first (their finish chain is
            # longer: MM -> ACT evac -> store), then the DVE chunks in
            # shrinking sizes so the final stt+store chain is short.
            pe_pieces = []
            for b in range(NB):
                c0 = W_V + b * BANK
                q = ppiece.tile([P, BANK], pe_dt, name=f"pq{b}", tag=f"pq{b}")
                ring = nc.sync if b % 2 == 0 else nc.scalar
                ring.dma_start(out=q[:], in_=dma_src(x_t[last][:, c0 : c0 + BANK]))
                pe_pieces.append(q)
            DV_CHUNKS = [768, 512, 256]
            assert sum(DV_CHUNKS) == W_V
            dv_pieces = []
            dv_slices = []
            c0 = 0
            for c, w in enumerate(DV_CHUNKS):
                q = ppiece.tile([P, w], f32, name=f"dq{c}", tag=f"dq{c}")
                ring = nc.sync if c % 2 == 0 else nc.scalar
                ring.dma_start(out=q[:], in_=x_t[last][:, c0 : c0 + w])
                dv_pieces.append(q)
                dv_slices.append(slice(c0, c0 + w))
                c0 += w

            # PE: per bank, plane 26's MM then the closing stop-MM, then
            # ScalarE evacuation with scale and store -- bank b's store
            # transfers while bank b+1 is still accumulating, filling the
            # DMA gap between the last loads and the store burst.
            for b in range(NB):
                c0 = W_V + b * BANK
                nc.tensor.matmul(
                    banks[b][:],
                    ident_sb[:],
                    tb26[:, b * BANK : (b + 1) * BANK],
                    start=False,
                    stop=False,
                )
                nc.tensor.matmul(
                    banks[b][:],
                    ident_sb[:],
                    pe_pieces[b][:],
                    start=False,
                    stop=True,
                )
                nc.scalar.mul(stage[b][:], banks[b][:], SCALE)
                nc.scalar.dma_start(out=out_t[:, c0 : c0 + BANK], in_=stage[b][:])

            # DVE: final fused (x*1/3 + scaled_acc) per chunk, store per
            # chunk.  dq0 (the big 384 KB chunk) rides the ACT ring behind
            # the stage stores; dq1/dq2 get the Sync ring so the final small
            # store never queues behind a big transfer.
            store_ring = [nc.scalar, nc.sync, nc.sync]
            for c, sl in enumerate(dv_slices):
                nc.vector.scalar_tensor_tensor(
                    acc[:, sl], dv_pieces[c][:], SCALE, acc[:, sl], mult, add
                )
                store_ring[c].dma_start(out=out_t[:, sl], in_=acc[:, sl])
    nc.compile()
    return nc


def _get_nc():
    global _NC_CACHE
    if _NC_CACHE is None:
        _NC_CACHE = _build_nc()
    return _NC_CACHE


_IDENT = np.eye(P, dtype=np.float32)


def _run(inputs, trace=False, trace_kwargs=None):
    e0 = np.asarray(inputs["expert_emb_0"], dtype=np.float32)
    e1 = np.asarray(inputs["expert_emb_1"], dtype=np.float32)
    e2 = np.asarray(inputs["expert_emb_2"], dtype=np.float32)
    B = e0.shape[1]
    assert B == N_CORES, f"expected B == {N_CORES}, got {B}"

    in_maps = []
    for b in range(B):
        xb = np.concatenate([e0[:, b], e1[:, b], e2[:, b]], axis=0)
        in_maps.append({"x": np.ascontiguousarray(xb), "ident": _IDENT})

    kw = {}
    if trace:
        kw["trace"] = True
        if trace_kwargs:
            kw.update(trace_kwargs)
    try:
        res = run_bass_kernel_spmd(_get_nc(), in_maps, list(range(N_CORES)), **kw)
    except Exception:
        # One retry: transient device errors (e.g. NRT unrecoverable after a
        # prior wedged run) usually clear on re-dispatch.
        res = run_bass_kernel_spmd(_get_nc(), in_maps, list(range(N_CORES)), **kw)
    out = np.stack([res.results[b]["out"] for b in range(B)], axis=0)
    return out.astype(np.float32, copy=False), res


def kernel(**inputs) -> np.ndarray:
    out, _ = _run(inputs, trace=False)
    return out


# revision 57
# speedup vs baseline: 1.1886x; 1.1886x over previous
"""DiffuseRouter kernel for 8 TRN2 NeuronCores.

Reference computation (enable_time=False, soft_time_routing=True):
    out[b, l, d] = (1/3) * sum_g sum_e expert_emb_g[e, b, l, d]
i.e. a uniform-weighted sum of 28 expert planes per batch element.

Sharding: pure data-parallel over batch B=8 -> one batch element per core.
Each core streams its 28 [256, 1280] f32 planes (36.7 MB) across both HWDGE
rings at ~415 GB/s and reduces them on the fly.  The reduction is split by
free-dim columns across two engines so neither falls behind the stream
(an all-DVE chain lags ~11 us):

  - cols [0:1536):    DVE chain (fp32 scalar_tensor_tensor, 1/3 folded in)
  - cols [1536:2560): TensorE identity-matmul accumulation into 2 PSUM
                      banks (fp32r moving operand), evacuated at the end by
                      ScalarE with the 1/3 scale.

Planes 0..25 load as single 1.31 MB transfers (best stream bandwidth);
plane 26 splits into DVE/PE stripes and plane 27 into five shrinking pieces
so each engine's final op and the output stores fire progressively as
pieces land.  Finer piecing of the last planes measures *worse*: DVE is the
serial resource at the tail and per-piece op overhead adds to its backlog.
"""

import numpy as np

import concourse.bacc as bacc
import concourse.tile as tile
from concourse import mybir
from concourse.alu_op_type import AluOpType
from concourse.bass_utils import run_bass_kernel_spmd

N_CORES = 8
E_TOTAL = 28  # 4 + 8 + 16 experts across the 3 granularity levels
L, D = 256, 1280
P = 128  # SBUF partitions
FD = (L // P) * D  # 2560 free-dim elements per partition
SCALE = 1.0 / 3.0

BANK = 512  # one PSUM bank = 512 fp32
W_V = 1536  # DVE stripe width (cols [0:W_V))
NB = (FD - W_V) // BANK  # PSUM banks for the PE stripe
PE_F32R = True  # fp32r moving operand (1 cyc/row) vs fp32 (4 cyc/row)

_NC_CACHE = None


def _build_nc():
    nc = bacc.Bacc(
        "TRN2", target_bir_lowering=False, debug=False, enable_partition_id=False
    )
    x = nc.dram_tensor("x", [E_TOTAL, L, D], mybir.dt.float32, kind="ExternalInput")
    ident = nc.dram_tensor("ident", [P, P], mybir.dt.float32, kind="ExternalInput")
    out = nc.dram_tensor("out", [L, D], mybir.dt.float32, kind="ExternalOutput")

    # [E, 256, 1280] -> [E, 128, 2560]: partition p holds rows 2p, 2p+1
    # (contiguous 10240 B per partition -> fully linear 1.31 MB DMA per plane).
    x_t = x.ap().rearrange("e (p a) d -> e p (a d)", a=2)
    out_t = out.ap().rearrange("(p a) d -> p (a d)", a=2)

    f32 = mybir.dt.float32
    f32r = mybir.dt.float32r
    mult, add = AluOpType.mult, AluOpType.add
    last = E_TOTAL - 1

    # walrus requires fp32r-matmul operands to be *produced* as fp32r, so the
    # plane tiles are declared fp32r (the DMA is a pure 4-byte copy either
    # way) and the DVE reads its stripe bitcast back to fp32.
    pe_dt = f32r if PE_F32R else f32

    def dma_src(ap):
        return ap.bitcast(f32r) if PE_F32R else ap

    with tile.TileContext(nc) as tc:
        with (
            tc.tile_pool(name="ina", bufs=2) as pina,
            tc.tile_pool(name="inb", bufs=8) as pinb,
            tc.tile_pool(name="pieces", bufs=1) as ppiece,
            tc.tile_pool(name="single", bufs=1) as psingle,
            tc.tile_pool(name="psum", bufs=1, space="PSUM") as ppsum,
        ):
            ident_sb = psingle.tile([P, P], pe_dt, name="ident", tag="ident")
            acc = psingle.tile([P, W_V], f32, name="acc", tag="acc")
            stage = [
                psingle.tile([P, BANK], f32, name=f"stage{b}", tag=f"stage{b}")
                for b in range(NB)
            ]
            banks = [
                ppsum.tile([P, BANK], f32, name=f"bank{b}", tag=f"bank{b}")
                for b in range(NB)
            ]

            # Identity for the PE accumulation; ACT ring so it never queues
            # ahead of plane loads on the SP ring.
            nc.scalar.dma_start(out=ident_sb[:], in_=dma_src(ident.ap()))

            for e in range(last):
                # Planes 0..25: one full 1.31 MB transfer (best stream BW).
                # Plane 26 splits into DVE-stripe + PE-stripe pieces so the
                # DVE add fires ~1.3 us earlier -- only the last planes'
                # latency matters for the tail.
                ring = nc.sync if e % 2 == 0 else nc.scalar
                if e < last - 1:
                    t = pinb.tile([P, FD], pe_dt)
                    ring.dma_start(out=t[:], in_=dma_src(x_t[e]))
                    ta = t[:, 0:W_V].bitcast(f32)
                    tb = t[:, W_V:FD]
                else:
                    ta_t = pina.tile([P, W_V], f32)
                    ring.dma_start(out=ta_t[:], in_=x_t[e][:, 0:W_V])
                    tb_t = pina.tile([P, FD - W_V], pe_dt)
                    ring.dma_start(out=tb_t[:], in_=dma_src(x_t[e][:, W_V:FD]))
                    ta, tb = ta_t[:], tb_t[:]
                # DVE stripe: scale folded into every op (stt costs the same
                # as a plain tensor_tensor add at fp32 1x mode), so nothing
                # but the per-chunk stt remains after the last plane lands.
                if e == 0:
                    nc.vector.tensor_scalar_mul(acc[:], ta, SCALE)
                else:
                    nc.vector.scalar_tensor_tensor(
                        acc[:], ta, SCALE, acc[:], mult, add
                    )
                # PE stripe: accumulate into PSUM banks
                for b in range(NB):
                    nc.tensor.matmul(
                        banks[b][:],
                        ident_sb[:],
                        tb[:, b * BANK : (b + 1) * BANK],
                        start=(e == 0),
                        stop=False,
                    )

            # Last plane in pieces: PE banks first (their finish chain is
            # longer: MM -> ACT evac -> store), then the DVE chunks in
            # shrinking sizes so the final stt+store chain is short.
            pe_pieces = []
            for b in range(NB):
                c0 = W_V + b * BANK
                q = ppiece.tile([P, BANK], pe_dt, name=f"pq{b}", tag=f"pq{b}")
                ring = nc.sync if b % 2 == 0 else nc.scalar
                ring.dma_start(out=q[:], in_=dma_src(x_t[last][:, c0 : c0 + BANK]))
                pe_pieces.append(q)
            DV_CHUNKS = [768, 512, 256]
            assert sum(DV_CHUNKS) == W_V
            dv_pieces = []
            dv_slices = []
            c0 = 0
            for c, w in enumerate(DV_CHUNKS):
                q = ppiece.tile([P, w], f32, name=f"dq{c}", tag=f"dq{c}")
                ring = nc.sync if c % 2 == 0 else nc.scalar
                ring.dma_start(out=q[:], in_=x_t[last][:, c0 : c0 + w])
                dv_pieces.append(q)
                dv_slices.append(slice(c0, c0 + w))
                c0 += w

            # PE: close each accumulation group, evacuate with scale, store.
            for b in range(NB):
                c0 = W_V + b * BANK
                nc.tensor.matmul(
                    banks[b][:],
                    ident_sb[:],
                    pe_pieces[b][:],
                    start=False,
                    stop=True,
                )
                nc.scalar.mul(stage[b][:], banks[b][:], SCALE)
                nc.scalar.dma_start(out=out_t[:, c0 : c0 + BANK], in_=stage[b][:])

            # DVE: final fused (x*1/3 + scaled_acc) per chunk, store per
            # chunk.  dq0 (the big 384 KB chunk) rides the ACT ring behind
            # the stage stores; dq1/dq2 get the Sync ring so the final small
            # store never queues behind a big transfer.
            store_ring = [nc.scalar, nc.sync, nc.sync]
            for c, sl in enumerate(dv_slices):
                nc.vector.scalar_tensor_tensor(
                    acc[:, sl], dv_pieces[c][:], SCALE, acc[:, sl], mult, add
                )
                store_ring[c].dma_start(out=out_t[:, sl], in_=acc[:, sl])
    nc.compile()
    return nc


def _get_nc():
    global _NC_CACHE
    if _NC_CACHE is None:
        _NC_CACHE = _build_nc()
    return _NC_CACHE


_IDENT = np.eye(P, dtype=np.float32)


def _run(inputs, trace=False, trace_kwargs=None):
    e0 = np.asarray(inputs["expert_emb_0"], dtype=np.float32)
    e1 = np.asarray(inputs["expert_emb_1"], dtype=np.float32)
    e2 = np.asarray(inputs["expert_emb_2"], dtype=np.float32)
    B = e0.shape[1]
    assert B == N_CORES, f"expected B == {N_CORES}, got {B}"

    in_maps = []
    for b in range(B):
        xb = np.concatenate([e0[:, b], e1[:, b], e2[:, b]], axis=0)
        in_maps.append({"x": np.ascontiguousarray(xb), "ident": _IDENT})

    kw = {}
    if trace:
        kw["trace"] = True
        if trace_kwargs:
            kw.update(trace_kwargs)
    try:
        res = run_bass_kernel_spmd(_get_nc(), in_maps, list(range(N_CORES)), **kw)
    except Exception:
        # One retry: transient device errors (e.g. NRT unrecoverable after a
        # prior wedged run) usually clear on re-dispatch.
        res = run_bass_kernel_spmd(_get_nc(), in_maps, list(range(N_CORES)), **kw)
    out = np.stack([res.results[b]["out"] for b in range(B)], axis=0)
    return out.astype(np.float32, copy=False), res


def kernel(**inputs) -> np.ndarray:
    out, _ = _run(inputs, trace=False)
    return out
